# revision 9
# baseline (speedup 1.0000x reference)
"""EpisodicMemory (DMN AttentionGRU) Trainium2 kernel.

Full-input contract: kernel(**inputs) takes the unsharded numpy inputs and
returns the full (128, 1, 1024) output. Internally shards batch 128 -> 8
cores x 16, runs a Bass/Tile kernel per core, gathers on host.

Per-core device program (B=16, S=128, H=1024), everything in an
(h-on-partitions, batch-on-free) layout, fp16 matmul inputs / fp32 psum:
  1. z = [f*q, f*m, |f-q|, |f-m|]  -> t = tanh(z @ Wz1.T + bz1)
  2. scores = t @ Wz2.T  -> G = softmax_s(scores)
  3. P = facts @ W.T + bw ; Q = facts @ Wr.T + br + bur
  4. 128-step AttnGRU scan:
       r = sigmoid(Q_t + C Ur^T); h = tanh(P_t + r*(C U^T + bu))
       C = C + g_t * (h - C)
  5. out = relu([prevM, C, questions] @ Wm.T + bm)
"""
import sys

sys.path.insert(0, "/opt/trn_rl_repo")

import numpy as np

import concourse.bass as bass
import concourse.tile as tile
import concourse.mybir as mybir
from concourse import bacc
from concourse.bass import ds
from concourse.bass_utils import run_bass_kernel_spmd
from concourse.masks import make_identity

F16 = mybir.dt.float16
F32 = mybir.dt.float32
AF = mybir.ActivationFunctionType

B, S, H = 128, 128, 1024
NCORES = 8
BL = B // NCORES  # 16 batch per core
HT = H // 128  # 8 h-tiles


def _ap(ap_obj, dims):
    """Rebuild an AP keeping/duplicating dims; entries are either an index
    into ap_obj.ap or a literal [stride, size] pair."""
    lst = []
    for d in dims:
        lst.append(list(ap_obj.ap[d]) if isinstance(d, int) else list(d))
    return bass.AP(tensor=ap_obj.tensor, offset=ap_obj.offset, ap=lst)


def _build():
    nc = bacc.Bacc("TRN2", target_bir_lowering=False, debug=False,
                   num_devices=NCORES)

    def inp(name, shape, dt=F16):
        return nc.dram_tensor(name, list(shape), dt, kind="ExternalInput").ap()

    facts_t = inp("facts_t", (128, HT, BL, S))
    q_t = inp("q_t", (128, HT, BL))
    m_t = inp("m_t", (128, HT, BL))
    wz1_t = inp("wz1_t", (128, 32, H))
    wz2_t = inp("wz2_t", (128, HT))
    w_t = inp("w_t", (128, HT, H))
    wr_t = inp("wr_t", (128, HT, H))
    ucat_t = inp("ucat_t", (128, HT, 2 * H))
    wm_t = inp("wm_t", (128, 24, H))
    bm_row = inp("bm_row", (1, H))
    bz1_c = inp("bz1_c", (128, HT), F32)
    qb_c = inp("qb_c", (128, HT), F32)
    pb_c = inp("pb_c", (128, HT), F32)
    bu_c = inp("bu_c", (128, HT, 1), F32)
    g_scr = nc.dram_tensor("g_scr", [BL, S], F32).ap()
    out = nc.dram_tensor("out", [BL, H], F32, kind="ExternalOutput").ap()

    with tile.TileContext(nc) as tc:
        with tc.tile_pool(name="persist", bufs=1) as pp, \
             tc.tile_pool(name="psum", bufs=4, space="PSUM") as ps, \
             tc.tile_pool(name="psumf", bufs=1, space="PSUM") as psf:
            # ---- long-lived tiles ----
            q_sb = pp.tile([128, HT, BL], F16)
            m_sb = pp.tile([128, HT, BL], F16)
            nc.sync.dma_start(out=q_sb, in_=q_t)
            nc.sync.dma_start(out=m_sb, in_=m_t)
            bz1_sb = pp.tile([128, HT], F32)
            qb_sb = pp.tile([128, HT], F32)
            pb_sb = pp.tile([128, HT], F32)
            bu_sb = pp.tile([128, HT, 1], F32)
            nc.sync.dma_start(out=bz1_sb, in_=bz1_c)
            nc.sync.dma_start(out=qb_sb, in_=qb_c)
            nc.sync.dma_start(out=pb_sb, in_=pb_c)
            nc.sync.dma_start(out=bu_sb, in_=bu_c)
            P_sb = pp.tile([128, HT, BL, S], F16)
            Q_sb = pp.tile([128, HT, BL, S], F16)
            G_sb = pp.tile([128, BL, S], F32)
            ident = pp.tile([128, 128], F32)
            make_identity(nc, ident)

            # ---- stage A: P, Q, gate MLP, scores, softmax ----
            with tc.tile_pool(name="st1", bufs=1) as p1, \
                 tc.tile_pool(name="st1d", bufs=2) as p1d, \
                 tc.tile_pool(name="st1t", bufs=3) as p1t:
                facts_sb = p1.tile([128, HT, BL, S], F16)
                nc.sync.dma_start(out=facts_sb, in_=facts_t)

                # P = facts@W.T + bw ; Q = facts@Wr.T + (br+bur)
                for (wt, dst, bias) in ((w_t, P_sb, pb_sb), (wr_t, Q_sb, qb_sb)):
                    for j in range(HT):
                        wj = p1d.tile([128, HT, 128], F16, tag="pqw")
                        nc.sync.dma_start(out=wj,
                                          in_=wt[:, :, j * 128:(j + 1) * 128])
                        for fc in range(4):
                            acc = ps.tile([128, 512], F32, tag="ps1")
                            for i in range(HT):
                                nc.tensor.matmul(
                                    acc, wj[:, i, :],
                                    facts_sb[:, i, 4 * fc:4 * fc + 4, :],
                                    start=(i == 0), stop=(i == HT - 1))
                            nc.scalar.activation(
                                out=dst[:, j, 4 * fc:4 * fc + 4, :], in_=acc,
                                func=AF.Identity, bias=bias[:, j:j + 1])

                # gate MLP with fused score accumulation
                wz2_sb = p1.tile([128, HT], F16)
                nc.sync.dma_start(out=wz2_sb, in_=wz2_t)
                psc = psf.tile([128, BL], F32, tag="psc")
                for fq in range(4):  # batch quarters of 4
                    bsl = slice(4 * fq, 4 * fq + 4)
                    z = p1d.tile([128, 32, 4, S], F16, tag="zq")
                    for blk in range(4):
                        vec = q_sb if blk in (0, 2) else m_sb
                        for hi in range(HT):
                            i = blk * HT + hi
                            fsl = facts_sb[:, hi, bsl, :]
                            vsl = _ap(vec[:, hi, bsl], [0, 1, [0, S]])
                            if blk < 2:
                                nc.vector.tensor_mul(z[:, i, :, :], fsl, vsl)
                            else:
                                d = p1t.tile([128, 4, S], F16, tag="zd")
                                nc.vector.tensor_sub(d, fsl, vsl)
                                nc.scalar.activation(out=z[:, i, :, :], in_=d,
                                                     func=AF.Abs)
                    for j in range(HT):
                        wz1j = p1d.tile([128, 32, 128], F16, tag="wz1j")
                        nc.sync.dma_start(out=wz1j,
                                          in_=wz1_t[:, :, j * 128:(j + 1) * 128])
                        acc = ps.tile([128, 512], F32, tag="ps1")
                        for i in range(32):
                            nc.tensor.matmul(acc, wz1j[:, i, :], z[:, i, :, :],
                                             start=(i == 0), stop=(i == 31))
                        tq = p1t.tile([128, 4, S], F16, tag="tq")
                        nc.scalar.activation(out=tq, in_=acc, func=AF.Tanh,
                                             bias=bz1_sb[:, j:j + 1])
                        for b in range(4):
                            nc.tensor.matmul(psc[:, 4 * fq + b:4 * fq + b + 1],
                                             tq[:, b, :], wz2_sb[:, j:j + 1],
                                             start=(j == 0), stop=(j == HT - 1))

                # softmax over s
                sc_sb = p1.tile([128, BL], F32)
                nc.vector.tensor_copy(sc_sb, psc)
                pst = ps.tile([BL, 128], F32, tag="ps1")
                nc.tensor.transpose(pst, sc_sb, ident)
                scT = p1.tile([BL, S], F32)
                nc.vector.tensor_copy(scT, pst)
                mx = p1.tile([BL, 1], F32)
                nc.vector.reduce_max(mx, scT, axis=mybir.AxisListType.X,
                                     negate=True)
                ex = p1.tile([BL, S], F32)
                nc.scalar.activation(out=ex, in_=scT, func=AF.Exp, bias=mx)
                sm = p1.tile([BL, 1], F32)
                nc.vector.reduce_sum(sm, ex, axis=mybir.AxisListType.X)
                rs = p1.tile([BL, 1], F32)
                nc.vector.reciprocal(rs, sm)
                g16 = p1.tile([BL, S], F32)
                nc.vector.tensor_scalar_mul(g16, ex, rs)
                nc.sync.dma_start(out=g_scr, in_=g16)
                # broadcast back: G_sb[p, b, s] = g[b, s]
                nc.sync.dma_start(out=G_sb,
                                  in_=g_scr[:, :].partition_broadcast(128))

            # ---- stage 4: scan ----
            lp_cm = tc.tile_pool(name="late", bufs=1)
            lp = lp_cm.__enter__()
            ucat_sb = lp.tile([128, HT, 2 * H], F16)
            nc.sync.dma_start(out=ucat_sb, in_=ucat_t)
            wm_sb = lp.tile([128, 24, H], F16)
            nc.sync.dma_start(out=wm_sb, in_=wm_t)
            bm_sb = lp.tile([1, H], F16)
            nc.sync.dma_start(out=bm_sb, in_=bm_row)
            ones_sb = lp.tile([1, BL], F16)
            nc.vector.memset(ones_sb, 1.0)

            C = pp.tile([128, HT, BL], F32)
            Cf = pp.tile([128, HT, BL], F16)
            nc.vector.memset(C, 0.0)
            nc.vector.memset(Cf, 0.0)

            with tc.tile_pool(name="scan", bufs=2) as sp:
                with tc.For_i(0, S, 1) as t:
                    acc = ps.tile([128, HT, 32], F32, tag="ps1")
                    # Ur block first (cols H..2H)
                    for j in range(HT):
                        for i in range(HT):
                            nc.tensor.matmul(
                                acc[:, j, 16:32],
                                ucat_sb[:, i, H + j * 128:H + (j + 1) * 128],
                                Cf[:, i, :],
                                start=(i == 0), stop=(i == HT - 1))
                    for j in range(HT):
                        for i in range(HT):
                            nc.tensor.matmul(
                                acc[:, j, 0:16],
                                ucat_sb[:, i, j * 128:(j + 1) * 128],
                                Cf[:, i, :],
                                start=(i == 0), stop=(i == HT - 1))
                    # Q_t, P_t, g_t slices: [p, j, b] at step t
                    qsl = _qp_slice(Q_sb, t)
                    psl = _qp_slice(P_sb, t)
                    rpre = sp.tile([128, HT, BL], F32, tag="rpre")
                    nc.vector.tensor_add(rpre, acc[:, :, 16:32], qsl)
                    r = sp.tile([128, HT, BL], F32, tag="r")
                    nc.scalar.activation(out=r, in_=rpre, func=AF.Sigmoid)
                    a = sp.tile([128, HT, BL], F32, tag="a")
                    nc.vector.tensor_add(a, acc[:, :, 0:16],
                                         bu_sb[:, :, :].to_broadcast(
                                             (128, HT, BL)))
                    ra = sp.tile([128, HT, BL], F32, tag="ra")
                    nc.vector.tensor_mul(ra, r, a)
                    hin = sp.tile([128, HT, BL], F32, tag="hin")
                    nc.vector.tensor_add(hin, ra, psl)
                    h = sp.tile([128, HT, BL], F32, tag="h")
                    nc.scalar.activation(out=h, in_=hin, func=AF.Tanh)
                    d = sp.tile([128, HT, BL], F32, tag="d")
                    nc.vector.tensor_sub(d, h, C)
                    gsl_raw = G_sb[:, :, ds(t, 1)]
                    gsl = _ap(gsl_raw, [0, [0, HT], 1])
                    dg = sp.tile([128, HT, BL], F32, tag="dg")
                    nc.vector.tensor_mul(dg, d, gsl)
                    nc.vector.tensor_add(C, C, dg)
                    nc.vector.tensor_copy(Cf, C)

            # ---- stage 5: final linear + relu ----
            accf = psf.tile([16, H], F32, tag="accf")
            for fc in range(2):
                fsl = slice(fc * 512, (fc + 1) * 512)
                for i in range(24):
                    lhs = (m_sb[:, i, :] if i < HT else
                           Cf[:, i - HT, :] if i < 2 * HT else
                           q_sb[:, i - 2 * HT, :])
                    nc.tensor.matmul(accf[:, fsl], lhs, wm_sb[:, i, fsl],
                                     start=(i == 0), stop=False)
                nc.tensor.matmul(accf[:, fsl], ones_sb, bm_sb[:, fsl],
                                 start=False, stop=True)
            outf = pp.tile([16, H], F32)
            nc.scalar.activation(out=outf, in_=accf, func=AF.Relu)
            nc.sync.dma_start(out=out, in_=outf)
            lp_cm.__exit__(None, None, None)

    nc.compile()
    return nc


def _qp_slice(t4, tvar):
    """t4 is (128, HT, BL, S); return (128, HT, BL) AP at s=tvar."""
    raw = t4[:, :, :, ds(tvar, 1)]
    return bass.AP(tensor=raw.tensor, offset=raw.offset,
                   ap=[list(raw.ap[0]), list(raw.ap[1]), list(raw.ap[2])])


_NC = None


def _get_nc():
    global _NC
    if _NC is None:
        _NC = _build()
    return _NC


def _prep_core(facts, questions, prevM, k):
    bsl = slice(k * BL, (k + 1) * BL)
    f = facts[bsl]  # (16, 128, 1024)
    # [p, i, b, s] = facts[b, s, i*128+p]
    ft = np.ascontiguousarray(
        f.transpose(2, 0, 1).reshape(HT, 128, BL, S).transpose(1, 0, 2, 3)
    ).astype(np.float16)
    q = questions[bsl, 0]  # (16, 1024)
    qt = np.ascontiguousarray(
        q.T.reshape(HT, 128, BL).transpose(1, 0, 2)).astype(np.float16)
    m = prevM[bsl, 0]
    mt = np.ascontiguousarray(
        m.T.reshape(HT, 128, BL).transpose(1, 0, 2)).astype(np.float16)
    return ft, qt, mt


def _prep_weights(Wr, br, Ur, bur, W, bw, U, bu, Wz1, bz1, Wz2, bz2, Wm, bm):
    def tl(wT, nt):  # (K, N) -> (128, nt, N) with K = nt*128
        K, N = wT.shape
        return np.ascontiguousarray(
            wT.reshape(nt, 128, N).transpose(1, 0, 2)).astype(np.float16)

    wz1_t = tl(np.ascontiguousarray(Wz1.T), 32)        # (4096,1024)
    wz2_t = np.ascontiguousarray(
        Wz2[0].reshape(HT, 128).T).astype(np.float16)  # (128, 8)
    w_t = tl(np.ascontiguousarray(W.T), HT)
    wr_t = tl(np.ascontiguousarray(Wr.T), HT)
    ucat = np.concatenate([U.T, Ur.T], axis=1)         # (1024, 2048)
    ucat_t = tl(np.ascontiguousarray(ucat), HT)
    wm_t = tl(np.ascontiguousarray(Wm.T), 24)          # (3072,1024)
    bm_row = bm.reshape(1, H).astype(np.float16)

    def cols(v):  # (1024,) -> (128, 8) [p, j]
        return np.ascontiguousarray(v.reshape(HT, 128).T).astype(np.float32)

    return dict(
        wz1_t=wz1_t, wz2_t=wz2_t, w_t=w_t, wr_t=wr_t, ucat_t=ucat_t,
        wm_t=wm_t, bm_row=bm_row, bz1_c=cols(bz1),
        qb_c=cols(br + bur), pb_c=cols(bw), bu_c=cols(bu)[:, :, None],
    )


def kernel(facts, questions, prevM, Wr, br, Ur, bur, W, bw, U, bu,
           Wz1, bz1, Wz2, bz2, Wm, bm):
    facts = np.asarray(facts, dtype=np.float32)
    questions = np.asarray(questions, dtype=np.float32)
    prevM = np.asarray(prevM, dtype=np.float32)
    wd = _prep_weights(np.asarray(Wr), np.asarray(br), np.asarray(Ur),
                       np.asarray(bur), np.asarray(W), np.asarray(bw),
                       np.asarray(U), np.asarray(bu), np.asarray(Wz1),
                       np.asarray(bz1), np.asarray(Wz2), np.asarray(bz2),
                       np.asarray(Wm), np.asarray(bm))
    in_maps = []
    for k in range(NCORES):
        ft, qt, mt = _prep_core(facts, questions, prevM, k)
        in_maps.append(dict(facts_t=ft, q_t=qt, m_t=mt, **wd))
    nc = _get_nc()
    res = run_bass_kernel_spmd(nc, in_maps, core_ids=list(range(NCORES)))
    outs = [res.results[k]["out"] for k in range(NCORES)]
    full = np.concatenate(outs, axis=0)  # (128, 1024)
    return full[:, None, :].astype(np.float32)


# revision 10
# speedup vs baseline: 1.0300x; 1.0300x over previous
"""EpisodicMemory (DMN AttentionGRU) Trainium2 kernel.

Full-input contract: kernel(**inputs) takes the unsharded numpy inputs and
returns the full (128, 1, 1024) output. Internally shards batch 128 -> 8
cores x 16, runs a Bass/Tile kernel per core, gathers on host.

Per-core device program (B=16, S=128, H=1024), everything in an
(h-on-partitions, batch-on-free) layout, fp16 matmul inputs / fp32 psum:
  1. z = [f*q, f*m, |f-q|, |f-m|]  -> t = tanh(z @ Wz1.T + bz1)
  2. scores = t @ Wz2.T  -> G = softmax_s(scores)
  3. P = facts @ W.T + bw ; Q = facts @ Wr.T + br + bur
  4. 128-step AttnGRU scan:
       r = sigmoid(Q_t + C Ur^T); h = tanh(P_t + r*(C U^T + bu))
       C = C + g_t * (h - C)
  5. out = relu([prevM, C, questions] @ Wm.T + bm)
"""
import sys

sys.path.insert(0, "/opt/trn_rl_repo")

import numpy as np

import concourse.bass as bass
import concourse.tile as tile
import concourse.mybir as mybir
from concourse import bacc
from concourse.bass import ds
from concourse.bass_utils import run_bass_kernel_spmd
from concourse.masks import make_identity

F16 = mybir.dt.float16
F32 = mybir.dt.float32
AF = mybir.ActivationFunctionType

B, S, H = 128, 128, 1024
NCORES = 8
BL = B // NCORES  # 16 batch per core
HT = H // 128  # 8 h-tiles


def _ap(ap_obj, dims):
    """Rebuild an AP keeping/duplicating dims; entries are either an index
    into ap_obj.ap or a literal [stride, size] pair."""
    lst = []
    for d in dims:
        lst.append(list(ap_obj.ap[d]) if isinstance(d, int) else list(d))
    return bass.AP(tensor=ap_obj.tensor, offset=ap_obj.offset, ap=lst)


def _build():
    nc = bacc.Bacc("TRN2", target_bir_lowering=False, debug=False,
                   num_devices=NCORES)

    def inp(name, shape, dt=F16):
        return nc.dram_tensor(name, list(shape), dt, kind="ExternalInput").ap()

    facts_t = inp("facts_t", (128, HT, BL, S))
    q_t = inp("q_t", (128, HT, BL))
    m_t = inp("m_t", (128, HT, BL))
    wz1_t = inp("wz1_t", (128, 32, H))
    wz2_t = inp("wz2_t", (128, HT))
    w_t = inp("w_t", (128, HT, H))
    wr_t = inp("wr_t", (128, HT, H))
    ucat_t = inp("ucat_t", (128, HT, 2 * H))
    wm_t = inp("wm_t", (128, 24, H))
    bm_row = inp("bm_row", (1, H))
    bz1_c = inp("bz1_c", (128, HT), F32)
    qb_c = inp("qb_c", (128, HT), F32)
    pb_c = inp("pb_c", (128, HT), F32)
    bu_c = inp("bu_c", (128, HT, 1), F32)
    g_scr = nc.dram_tensor("g_scr", [BL, S], F32).ap()
    out = nc.dram_tensor("out", [BL, H], F32, kind="ExternalOutput").ap()

    with tile.TileContext(nc) as tc:
        with tc.tile_pool(name="persist", bufs=1) as pp, \
             tc.tile_pool(name="psum", bufs=4, space="PSUM") as ps, \
             tc.tile_pool(name="psumf", bufs=1, space="PSUM") as psf:
            # ---- long-lived tiles ----
            q_sb = pp.tile([128, HT, BL], F16)
            m_sb = pp.tile([128, HT, BL], F16)
            nc.sync.dma_start(out=q_sb, in_=q_t)
            nc.sync.dma_start(out=m_sb, in_=m_t)
            bz1_sb = pp.tile([128, HT], F32)
            qb_sb = pp.tile([128, HT], F32)
            pb_sb = pp.tile([128, HT], F32)
            bu_sb = pp.tile([128, HT, 1], F32)
            nc.sync.dma_start(out=bz1_sb, in_=bz1_c)
            nc.sync.dma_start(out=qb_sb, in_=qb_c)
            nc.sync.dma_start(out=pb_sb, in_=pb_c)
            nc.sync.dma_start(out=bu_sb, in_=bu_c)
            P_sb = pp.tile([128, HT, BL, S], F16)
            Q_sb = pp.tile([128, HT, BL, S], F16)
            G_sb = pp.tile([128, BL, S], F32)
            ident = pp.tile([128, 128], F32)
            make_identity(nc, ident)

            # ---- stage A: P, Q, gate MLP, scores, softmax ----
            with tc.tile_pool(name="st1", bufs=1) as p1, \
                 tc.tile_pool(name="st1d", bufs=2) as p1d, \
                 tc.tile_pool(name="st1t", bufs=3) as p1t:
                facts_sb = p1.tile([128, HT, BL, S], F16)
                nc.sync.dma_start(out=facts_sb, in_=facts_t)

                # P = facts@W.T + bw ; Q = facts@Wr.T + (br+bur)
                for (wt, dst, bias) in ((w_t, P_sb, pb_sb), (wr_t, Q_sb, qb_sb)):
                    for j in range(HT):
                        wj = p1d.tile([128, HT, 128], F16, tag="pqw")
                        nc.sync.dma_start(out=wj,
                                          in_=wt[:, :, j * 128:(j + 1) * 128])
                        for fc in range(4):
                            acc = ps.tile([128, 512], F32, tag="ps1")
                            for i in range(HT):
                                nc.tensor.matmul(
                                    acc, wj[:, i, :],
                                    facts_sb[:, i, 4 * fc:4 * fc + 4, :],
                                    start=(i == 0), stop=(i == HT - 1))
                            nc.scalar.activation(
                                out=dst[:, j, 4 * fc:4 * fc + 4, :], in_=acc,
                                func=AF.Identity, bias=bias[:, j:j + 1])

                # gate MLP with fused score accumulation
                wz2_sb = p1.tile([128, HT], F16)
                nc.sync.dma_start(out=wz2_sb, in_=wz2_t)
                psc = psf.tile([128, BL], F32, tag="psc")
                for fq in range(4):  # batch quarters of 4
                    bsl = slice(4 * fq, 4 * fq + 4)
                    z = p1d.tile([128, 32, 4, S], F16, tag="zq")
                    for blk in range(4):
                        vec = q_sb if blk in (0, 2) else m_sb
                        for hi in range(HT):
                            i = blk * HT + hi
                            fsl = facts_sb[:, hi, bsl, :]
                            vsl = _ap(vec[:, hi, bsl], [0, 1, [0, S]])
                            if blk < 2:
                                nc.vector.tensor_mul(z[:, i, :, :], fsl, vsl)
                            else:
                                d = p1t.tile([128, 4, S], F16, tag="zd")
                                nc.vector.tensor_sub(d, fsl, vsl)
                                nc.scalar.activation(out=z[:, i, :, :], in_=d,
                                                     func=AF.Abs)
                    for j in range(HT):
                        wz1j = p1d.tile([128, 32, 128], F16, tag="wz1j")
                        nc.sync.dma_start(out=wz1j,
                                          in_=wz1_t[:, :, j * 128:(j + 1) * 128])
                        acc = ps.tile([128, 512], F32, tag="ps1")
                        for i in range(32):
                            nc.tensor.matmul(acc, wz1j[:, i, :], z[:, i, :, :],
                                             start=(i == 0), stop=(i == 31))
                        tq = p1t.tile([128, 4, S], F16, tag="tq")
                        nc.scalar.activation(out=tq, in_=acc, func=AF.Tanh,
                                             bias=bz1_sb[:, j:j + 1])
                        for b in range(4):
                            nc.tensor.matmul(psc[:, 4 * fq + b:4 * fq + b + 1],
                                             tq[:, b, :], wz2_sb[:, j:j + 1],
                                             start=(j == 0), stop=(j == HT - 1))

                # softmax over s
                sc_sb = p1.tile([128, BL], F32)
                nc.vector.tensor_copy(sc_sb, psc)
                pst = ps.tile([BL, 128], F32, tag="ps1")
                nc.tensor.transpose(pst, sc_sb, ident)
                scT = p1.tile([BL, S], F32)
                nc.vector.tensor_copy(scT, pst)
                mx = p1.tile([BL, 1], F32)
                nc.vector.reduce_max(mx, scT, axis=mybir.AxisListType.X,
                                     negate=True)
                ex = p1.tile([BL, S], F32)
                nc.scalar.activation(out=ex, in_=scT, func=AF.Exp, bias=mx)
                sm = p1.tile([BL, 1], F32)
                nc.vector.reduce_sum(sm, ex, axis=mybir.AxisListType.X)
                rs = p1.tile([BL, 1], F32)
                nc.vector.reciprocal(rs, sm)
                g16 = p1.tile([BL, S], F32)
                nc.vector.tensor_scalar_mul(g16, ex, rs)
                nc.sync.dma_start(out=g_scr, in_=g16)
                # broadcast back: G_sb[p, b, s] = g[b, s]
                nc.sync.dma_start(out=G_sb,
                                  in_=g_scr[:, :].partition_broadcast(128))

            # ---- stage 4: scan ----
            lp_cm = tc.tile_pool(name="late", bufs=1)
            lp = lp_cm.__enter__()
            ucat_sb = lp.tile([128, HT, 2 * H], F16)
            nc.sync.dma_start(out=ucat_sb, in_=ucat_t)
            wm_sb = lp.tile([128, 24, H], F16)
            nc.sync.dma_start(out=wm_sb, in_=wm_t)
            bm_sb = lp.tile([1, H], F16)
            nc.sync.dma_start(out=bm_sb, in_=bm_row)
            ones_sb = lp.tile([1, BL], F16)
            nc.vector.memset(ones_sb, 1.0)

            C = pp.tile([128, HT, BL], F32)
            Cf = pp.tile([128, HT, BL], F16)
            nc.vector.memset(C, 0.0)
            nc.vector.memset(Cf, 0.0)

            def scan_step(sp, tv):
                acc = ps.tile([128, HT, 32], F32, tag="ps1")
                # Ur block first (cols H..2H): r-chain overlaps U block MMs
                for j in range(HT):
                    for i in range(HT):
                        nc.tensor.matmul(
                            acc[:, j, 16:32],
                            ucat_sb[:, i, H + j * 128:H + (j + 1) * 128],
                            Cf[:, i, :],
                            start=(i == 0), stop=(i == HT - 1))
                for j in range(HT):
                    for i in range(HT):
                        nc.tensor.matmul(
                            acc[:, j, 0:16],
                            ucat_sb[:, i, j * 128:(j + 1) * 128],
                            Cf[:, i, :],
                            start=(i == 0), stop=(i == HT - 1))
                qsl = _qp_slice(Q_sb, tv)
                psl = _qp_slice(P_sb, tv)
                rpre = sp.tile([128, HT, BL], F32, tag="rpre")
                nc.vector.tensor_add(rpre, acc[:, :, 16:32], qsl)
                r = sp.tile([128, HT, BL], F32, tag="r")
                nc.scalar.activation(out=r, in_=rpre, func=AF.Sigmoid)
                a = sp.tile([128, HT, BL], F32, tag="a")
                nc.vector.tensor_add(a, acc[:, :, 0:16],
                                     bu_sb[:, :, :].to_broadcast(
                                         (128, HT, BL)))
                ra = sp.tile([128, HT, BL], F32, tag="ra")
                nc.vector.tensor_mul(ra, r, a)
                hin = sp.tile([128, HT, BL], F32, tag="hin")
                nc.vector.tensor_add(hin, ra, psl)
                h = sp.tile([128, HT, BL], F32, tag="h")
                nc.scalar.activation(out=h, in_=hin, func=AF.Tanh)
                d = sp.tile([128, HT, BL], F32, tag="d")
                nc.vector.tensor_sub(d, h, C)
                gsl_raw = G_sb[:, :, ds(tv, 1)]
                gsl = _ap(gsl_raw, [0, [0, HT], 1])
                dg = sp.tile([128, HT, BL], F32, tag="dg")
                nc.vector.tensor_mul(dg, d, gsl)
                nc.vector.tensor_add(C, C, dg)
                nc.vector.tensor_copy(Cf, C)

            with tc.tile_pool(name="scan", bufs=2) as sp:
                with tc.For_i(0, S, 2, staggered_reset=True) as t:
                    scan_step(sp, t)
                    scan_step(sp, t + 1)

            # ---- stage 5: final linear + relu ----
            accf = psf.tile([16, H], F32, tag="accf")
            for fc in range(2):
                fsl = slice(fc * 512, (fc + 1) * 512)
                for i in range(24):
                    lhs = (m_sb[:, i, :] if i < HT else
                           Cf[:, i - HT, :] if i < 2 * HT else
                           q_sb[:, i - 2 * HT, :])
                    nc.tensor.matmul(accf[:, fsl], lhs, wm_sb[:, i, fsl],
                                     start=(i == 0), stop=False)
                nc.tensor.matmul(accf[:, fsl], ones_sb, bm_sb[:, fsl],
                                 start=False, stop=True)
            outf = pp.tile([16, H], F32)
            nc.scalar.activation(out=outf, in_=accf, func=AF.Relu)
            nc.sync.dma_start(out=out, in_=outf)
            lp_cm.__exit__(None, None, None)

    nc.compile()
    return nc


def _qp_slice(t4, tvar):
    """t4 is (128, HT, BL, S); return (128, HT, BL) AP at s=tvar."""
    raw = t4[:, :, :, ds(tvar, 1)]
    return bass.AP(tensor=raw.tensor, offset=raw.offset,
                   ap=[list(raw.ap[0]), list(raw.ap[1]), list(raw.ap[2])])


_NC = None


def _get_nc():
    global _NC
    if _NC is None:
        _NC = _build()
    return _NC


def _prep_core(facts, questions, prevM, k):
    bsl = slice(k * BL, (k + 1) * BL)
    f = facts[bsl]  # (16, 128, 1024)
    # [p, i, b, s] = facts[b, s, i*128+p]
    ft = np.ascontiguousarray(
        f.transpose(2, 0, 1).reshape(HT, 128, BL, S).transpose(1, 0, 2, 3)
    ).astype(np.float16)
    q = questions[bsl, 0]  # (16, 1024)
    qt = np.ascontiguousarray(
        q.T.reshape(HT, 128, BL).transpose(1, 0, 2)).astype(np.float16)
    m = prevM[bsl, 0]
    mt = np.ascontiguousarray(
        m.T.reshape(HT, 128, BL).transpose(1, 0, 2)).astype(np.float16)
    return ft, qt, mt


def _prep_weights(Wr, br, Ur, bur, W, bw, U, bu, Wz1, bz1, Wz2, bz2, Wm, bm):
    def tl(wT, nt):  # (K, N) -> (128, nt, N) with K = nt*128
        K, N = wT.shape
        return np.ascontiguousarray(
            wT.reshape(nt, 128, N).transpose(1, 0, 2)).astype(np.float16)

    wz1_t = tl(np.ascontiguousarray(Wz1.T), 32)        # (4096,1024)
    wz2_t = np.ascontiguousarray(
        Wz2[0].reshape(HT, 128).T).astype(np.float16)  # (128, 8)
    w_t = tl(np.ascontiguousarray(W.T), HT)
    wr_t = tl(np.ascontiguousarray(Wr.T), HT)
    ucat = np.concatenate([U.T, Ur.T], axis=1)         # (1024, 2048)
    ucat_t = tl(np.ascontiguousarray(ucat), HT)
    wm_t = tl(np.ascontiguousarray(Wm.T), 24)          # (3072,1024)
    bm_row = bm.reshape(1, H).astype(np.float16)

    def cols(v):  # (1024,) -> (128, 8) [p, j]
        return np.ascontiguousarray(v.reshape(HT, 128).T).astype(np.float32)

    return dict(
        wz1_t=wz1_t, wz2_t=wz2_t, w_t=w_t, wr_t=wr_t, ucat_t=ucat_t,
        wm_t=wm_t, bm_row=bm_row, bz1_c=cols(bz1),
        qb_c=cols(br + bur), pb_c=cols(bw), bu_c=cols(bu)[:, :, None],
    )


def kernel(facts, questions, prevM, Wr, br, Ur, bur, W, bw, U, bu,
           Wz1, bz1, Wz2, bz2, Wm, bm):
    facts = np.asarray(facts, dtype=np.float32)
    questions = np.asarray(questions, dtype=np.float32)
    prevM = np.asarray(prevM, dtype=np.float32)
    wd = _prep_weights(np.asarray(Wr), np.asarray(br), np.asarray(Ur),
                       np.asarray(bur), np.asarray(W), np.asarray(bw),
                       np.asarray(U), np.asarray(bu), np.asarray(Wz1),
                       np.asarray(bz1), np.asarray(Wz2), np.asarray(bz2),
                       np.asarray(Wm), np.asarray(bm))
    in_maps = []
    for k in range(NCORES):
        ft, qt, mt = _prep_core(facts, questions, prevM, k)
        in_maps.append(dict(facts_t=ft, q_t=qt, m_t=mt, **wd))
    nc = _get_nc()
    res = run_bass_kernel_spmd(nc, in_maps, core_ids=list(range(NCORES)))
    outs = [res.results[k]["out"] for k in range(NCORES)]
    full = np.concatenate(outs, axis=0)  # (128, 1024)
    return full[:, None, :].astype(np.float32)


# revision 12
# speedup vs baseline: 1.1333x; 1.1003x over previous
"""EpisodicMemory (DMN AttentionGRU) Trainium2 kernel.

Full-input contract: kernel(**inputs) takes the unsharded numpy inputs and
returns the full (128, 1, 1024) output. Internally shards batch 128 -> 8
cores x 16, runs a Bass/Tile kernel per core, gathers on host.

Per-core device program (B=16, S=128, H=1024), everything in an
(h-on-partitions, batch-on-free) layout, fp16 matmul inputs / fp32 psum:
  1. z = [f*q, f*m, |f-q|, |f-m|]  -> t = tanh(z @ Wz1.T + bz1)
  2. scores = t @ Wz2.T  -> G = softmax_s(scores)
  3. P = facts @ W.T + bw ; Q = facts @ Wr.T + br + bur
  4. 128-step AttnGRU scan:
       r = sigmoid(Q_t + C Ur^T); h = tanh(P_t + r*(C U^T + bu))
       C = C + g_t * (h - C)
  5. out = relu([prevM, C, questions] @ Wm.T + bm)
"""
import sys

sys.path.insert(0, "/opt/trn_rl_repo")

import numpy as np

import concourse.bass as bass
import concourse.tile as tile
import concourse.mybir as mybir
from concourse import bacc
from concourse.bass import ds
from concourse.bass_utils import run_bass_kernel_spmd
from concourse.masks import make_identity

F16 = mybir.dt.float16
F32 = mybir.dt.float32
AF = mybir.ActivationFunctionType

B, S, H = 128, 128, 1024
SCAN_REPEAT = 1
NCORES = 8
BL = B // NCORES  # 16 batch per core
HT = H // 128  # 8 h-tiles


def _ap(ap_obj, dims):
    """Rebuild an AP keeping/duplicating dims; entries are either an index
    into ap_obj.ap or a literal [stride, size] pair."""
    lst = []
    for d in dims:
        lst.append(list(ap_obj.ap[d]) if isinstance(d, int) else list(d))
    return bass.AP(tensor=ap_obj.tensor, offset=ap_obj.offset, ap=lst)


def _build():
    nc = bacc.Bacc("TRN2", target_bir_lowering=False, debug=False,
                   num_devices=NCORES)

    def inp(name, shape, dt=F16):
        return nc.dram_tensor(name, list(shape), dt, kind="ExternalInput").ap()

    facts_t = inp("facts_t", (128, HT, BL, S))
    q_t = inp("q_t", (128, HT, BL))
    m_t = inp("m_t", (128, HT, BL))
    wz1_t = inp("wz1_t", (128, 32, H))
    wz2_t = inp("wz2_t", (128, HT))
    w_t = inp("w_t", (128, HT, H))
    wr_t = inp("wr_t", (128, HT, H))
    ucat_t = inp("ucat_t", (128, HT, 2 * H))
    wm_t = inp("wm_t", (128, 24, H))
    bm_row = inp("bm_row", (1, H))
    bz1_c = inp("bz1_c", (128, HT), F32)
    qb_c = inp("qb_c", (128, HT), F32)
    pb_c = inp("pb_c", (128, HT), F32)
    bu_c = inp("bu_c", (128, HT, 1), F32)
    g_scr = nc.dram_tensor("g_scr", [BL, S], F32).ap()
    out = nc.dram_tensor("out", [BL, H], F32, kind="ExternalOutput").ap()

    with tile.TileContext(nc) as tc:
        with tc.tile_pool(name="persist", bufs=1) as pp, \
             tc.tile_pool(name="psum", bufs=4, space="PSUM") as ps, \
             tc.tile_pool(name="psumf", bufs=1, space="PSUM") as psf:
            # ---- long-lived tiles ----
            q_sb = pp.tile([128, HT, BL], F16)
            m_sb = pp.tile([128, HT, BL], F16)
            nc.sync.dma_start(out=q_sb, in_=q_t)
            nc.sync.dma_start(out=m_sb, in_=m_t)
            bz1_sb = pp.tile([128, HT], F32)
            qb_sb = pp.tile([128, HT], F32)
            pb_sb = pp.tile([128, HT], F32)
            bu_sb = pp.tile([128, HT, 1], F32)
            nc.sync.dma_start(out=bz1_sb, in_=bz1_c)
            nc.sync.dma_start(out=qb_sb, in_=qb_c)
            nc.sync.dma_start(out=pb_sb, in_=pb_c)
            nc.sync.dma_start(out=bu_sb, in_=bu_c)
            P_sb = pp.tile([128, HT, BL, S], F16)
            Q_sb = pp.tile([128, HT, BL, S], F16)
            G_sb = pp.tile([128, BL, S], F32)
            ident = pp.tile([128, 128], F32)
            make_identity(nc, ident)

            # ---- stage A: P, Q, gate MLP, scores, softmax ----
            with tc.tile_pool(name="st1", bufs=1) as p1, \
                 tc.tile_pool(name="st1d", bufs=2) as p1d, \
                 tc.tile_pool(name="st1t", bufs=3) as p1t:
                facts_sb = p1.tile([128, HT, BL, S], F16)
                nc.sync.dma_start(out=facts_sb, in_=facts_t)

                # P = facts@W.T + bw ; Q = facts@Wr.T + (br+bur)
                for (wt, dst, bias) in ((w_t, P_sb, pb_sb), (wr_t, Q_sb, qb_sb)):
                    for j in range(HT):
                        wj = p1d.tile([128, HT, 128], F16, tag="pqw")
                        nc.sync.dma_start(out=wj,
                                          in_=wt[:, :, j * 128:(j + 1) * 128])
                        for fc in range(4):
                            acc = ps.tile([128, 512], F32, tag="ps1")
                            for i in range(HT):
                                nc.tensor.matmul(
                                    acc, wj[:, i, :],
                                    facts_sb[:, i, 4 * fc:4 * fc + 4, :],
                                    start=(i == 0), stop=(i == HT - 1))
                            nc.scalar.activation(
                                out=dst[:, j, 4 * fc:4 * fc + 4, :], in_=acc,
                                func=AF.Identity, bias=bias[:, j:j + 1])

                # gate MLP with fused score accumulation
                wz2_sb = p1.tile([128, HT], F16)
                nc.sync.dma_start(out=wz2_sb, in_=wz2_t)
                psc = psf.tile([128, BL], F32, tag="psc")
                for fq in range(4):  # batch quarters of 4
                    bsl = slice(4 * fq, 4 * fq + 4)
                    z = p1d.tile([128, 32, 4, S], F16, tag="zq")
                    for blk in range(4):
                        vec = q_sb if blk in (0, 2) else m_sb
                        for hi in range(HT):
                            i = blk * HT + hi
                            fsl = facts_sb[:, hi, bsl, :]
                            vsl = _ap(vec[:, hi, bsl], [0, 1, [0, S]])
                            if blk < 2:
                                nc.vector.tensor_mul(z[:, i, :, :], fsl, vsl)
                            else:
                                d = p1t.tile([128, 4, S], F16, tag="zd")
                                nc.vector.tensor_sub(d, fsl, vsl)
                                nc.scalar.activation(out=z[:, i, :, :], in_=d,
                                                     func=AF.Abs)
                    for j in range(HT):
                        wz1j = p1d.tile([128, 32, 128], F16, tag="wz1j")
                        nc.sync.dma_start(out=wz1j,
                                          in_=wz1_t[:, :, j * 128:(j + 1) * 128])
                        acc = ps.tile([128, 512], F32, tag="ps1")
                        for i in range(32):
                            nc.tensor.matmul(acc, wz1j[:, i, :], z[:, i, :, :],
                                             start=(i == 0), stop=(i == 31))
                        tq = p1t.tile([128, 4, S], F16, tag="tq")
                        nc.scalar.activation(out=tq, in_=acc, func=AF.Tanh,
                                             bias=bz1_sb[:, j:j + 1])
                        for b in range(4):
                            nc.tensor.matmul(psc[:, 4 * fq + b:4 * fq + b + 1],
                                             tq[:, b, :], wz2_sb[:, j:j + 1],
                                             start=(j == 0), stop=(j == HT - 1))

                # softmax over s
                sc_sb = p1.tile([128, BL], F32)
                nc.vector.tensor_copy(sc_sb, psc)
                pst = ps.tile([BL, 128], F32, tag="ps1")
                nc.tensor.transpose(pst, sc_sb, ident)
                scT = p1.tile([BL, S], F32)
                nc.vector.tensor_copy(scT, pst)
                mx = p1.tile([BL, 1], F32)
                nc.vector.reduce_max(mx, scT, axis=mybir.AxisListType.X,
                                     negate=True)
                ex = p1.tile([BL, S], F32)
                nc.scalar.activation(out=ex, in_=scT, func=AF.Exp, bias=mx)
                sm = p1.tile([BL, 1], F32)
                nc.vector.reduce_sum(sm, ex, axis=mybir.AxisListType.X)
                rs = p1.tile([BL, 1], F32)
                nc.vector.reciprocal(rs, sm)
                g16 = p1.tile([BL, S], F32)
                nc.vector.tensor_scalar_mul(g16, ex, rs)
                nc.sync.dma_start(out=g_scr, in_=g16)
                # broadcast back: G_sb[p, b, s] = g[b, s]
                nc.sync.dma_start(out=G_sb,
                                  in_=g_scr[:, :].partition_broadcast(128))

            # ---- stage 4: scan ----
            lp_cm = tc.tile_pool(name="late", bufs=1)
            lp = lp_cm.__enter__()
            ucat_sb = lp.tile([128, HT, 2 * H], F16)
            nc.sync.dma_start(out=ucat_sb, in_=ucat_t)
            wm_sb = lp.tile([128, 24, H], F16)
            nc.sync.dma_start(out=wm_sb, in_=wm_t)
            bm_sb = lp.tile([1, H], F16)
            nc.sync.dma_start(out=bm_sb, in_=bm_row)
            ones_sb = lp.tile([1, BL], F16)
            nc.vector.memset(ones_sb, 1.0)

            C = pp.tile([128, HT, BL], F32)
            Cf = pp.tile([128, HT, BL], F16)
            nc.vector.memset(C, 0.0)
            nc.vector.memset(Cf, 0.0)

            def scan_step(sp, tv):
                acc = ps.tile([128, HT, 32], F32, tag="ps1")
                # Ur block first (cols H..2H): r-chain overlaps U block MMs
                for j in range(HT):
                    for i in range(HT):
                        nc.tensor.matmul(
                            acc[:, j, 16:32],
                            ucat_sb[:, i, H + j * 128:H + (j + 1) * 128],
                            Cf[:, i, :],
                            start=(i == 0), stop=(i == HT - 1))
                for j in range(HT):
                    for i in range(HT):
                        nc.tensor.matmul(
                            acc[:, j, 0:16],
                            ucat_sb[:, i, j * 128:(j + 1) * 128],
                            Cf[:, i, :],
                            start=(i == 0), stop=(i == HT - 1))
                qsl = _qp_slice(Q_sb, tv)
                psl = _qp_slice(P_sb, tv)
                rpre = sp.tile([128, HT, BL], F32, tag="rpre")
                nc.vector.tensor_add(rpre, acc[:, :, 16:32], qsl)
                r = sp.tile([128, HT, BL], F32, tag="r")
                nc.scalar.activation(out=r, in_=rpre, func=AF.Sigmoid)
                a = sp.tile([128, HT, BL], F32, tag="a")
                nc.vector.tensor_add(a, acc[:, :, 0:16],
                                     bu_sb[:, :, :].to_broadcast(
                                         (128, HT, BL)))
                ra = sp.tile([128, HT, BL], F32, tag="ra")
                nc.vector.tensor_mul(ra, r, a)
                hin = sp.tile([128, HT, BL], F32, tag="hin")
                nc.vector.tensor_add(hin, ra, psl)
                h = sp.tile([128, HT, BL], F32, tag="h")
                nc.scalar.activation(out=h, in_=hin, func=AF.Tanh)
                d = sp.tile([128, HT, BL], F32, tag="d")
                nc.vector.tensor_sub(d, h, C)
                gsl_raw = G_sb[:, :, ds(tv, 1)]
                gsl = _ap(gsl_raw, [0, [0, HT], 1])
                dg = sp.tile([128, HT, BL], F32, tag="dg")
                nc.vector.tensor_mul(dg, d, gsl)
                nc.vector.tensor_add(C, C, dg)
                nc.vector.tensor_copy(Cf, C)

            with tc.tile_pool(name="scan", bufs=2) as sp:
                with tc.For_i(0, SCAN_REPEAT, 1):
                    with tc.For_i(0, S, 2, staggered_reset=True) as t:
                        scan_step(sp, t)
                        scan_step(sp, t + 1)

            # ---- stage 5: final linear + relu ----
            accf = psf.tile([16, H], F32, tag="accf")
            for fc in range(2):
                fsl = slice(fc * 512, (fc + 1) * 512)
                for i in range(24):
                    lhs = (m_sb[:, i, :] if i < HT else
                           Cf[:, i - HT, :] if i < 2 * HT else
                           q_sb[:, i - 2 * HT, :])
                    nc.tensor.matmul(accf[:, fsl], lhs, wm_sb[:, i, fsl],
                                     start=(i == 0), stop=False)
                nc.tensor.matmul(accf[:, fsl], ones_sb, bm_sb[:, fsl],
                                 start=False, stop=True)
            outf = pp.tile([16, H], F32)
            nc.scalar.activation(out=outf, in_=accf, func=AF.Relu)
            nc.sync.dma_start(out=out, in_=outf)
            lp_cm.__exit__(None, None, None)

    nc.compile()
    return nc


def _qp_slice(t4, tvar):
    """t4 is (128, HT, BL, S); return (128, HT, BL) AP at s=tvar."""
    raw = t4[:, :, :, ds(tvar, 1)]
    return bass.AP(tensor=raw.tensor, offset=raw.offset,
                   ap=[list(raw.ap[0]), list(raw.ap[1]), list(raw.ap[2])])


_NC = None
_WCACHE = {}


def _get_nc():
    global _NC
    if _NC is None:
        _NC = _build()
    return _NC


def _prep_core(facts, questions, prevM, k):
    bsl = slice(k * BL, (k + 1) * BL)
    f = facts[bsl]  # (16, 128, 1024)
    # [p, i, b, s] = facts[b, s, i*128+p]
    ft = np.ascontiguousarray(
        f.transpose(2, 0, 1).reshape(HT, 128, BL, S).transpose(1, 0, 2, 3)
    ).astype(np.float16)
    q = questions[bsl, 0]  # (16, 1024)
    qt = np.ascontiguousarray(
        q.T.reshape(HT, 128, BL).transpose(1, 0, 2)).astype(np.float16)
    m = prevM[bsl, 0]
    mt = np.ascontiguousarray(
        m.T.reshape(HT, 128, BL).transpose(1, 0, 2)).astype(np.float16)
    return ft, qt, mt


def _prep_weights(Wr, br, Ur, bur, W, bw, U, bu, Wz1, bz1, Wz2, bz2, Wm, bm):
    def tl(wT, nt):  # (K, N) -> (128, nt, N) with K = nt*128
        K, N = wT.shape
        return np.ascontiguousarray(
            wT.reshape(nt, 128, N).transpose(1, 0, 2)).astype(np.float16)

    wz1_t = tl(np.ascontiguousarray(Wz1.T), 32)        # (4096,1024)
    wz2_t = np.ascontiguousarray(
        Wz2[0].reshape(HT, 128).T).astype(np.float16)  # (128, 8)
    w_t = tl(np.ascontiguousarray(W.T), HT)
    wr_t = tl(np.ascontiguousarray(Wr.T), HT)
    ucat = np.concatenate([U.T, Ur.T], axis=1)         # (1024, 2048)
    ucat_t = tl(np.ascontiguousarray(ucat), HT)
    wm_t = tl(np.ascontiguousarray(Wm.T), 24)          # (3072,1024)
    bm_row = bm.reshape(1, H).astype(np.float16)

    def cols(v):  # (1024,) -> (128, 8) [p, j]
        return np.ascontiguousarray(v.reshape(HT, 128).T).astype(np.float32)

    return dict(
        wz1_t=wz1_t, wz2_t=wz2_t, w_t=w_t, wr_t=wr_t, ucat_t=ucat_t,
        wm_t=wm_t, bm_row=bm_row, bz1_c=cols(bz1),
        qb_c=cols(br + bur), pb_c=cols(bw), bu_c=cols(bu)[:, :, None],
    )


def kernel(facts, questions, prevM, Wr, br, Ur, bur, W, bw, U, bu,
           Wz1, bz1, Wz2, bz2, Wm, bm):
    facts = np.asarray(facts, dtype=np.float32)
    questions = np.asarray(questions, dtype=np.float32)
    prevM = np.asarray(prevM, dtype=np.float32)
    wkey = (id(Wr), id(Ur), id(W), id(U), id(Wz1), id(Wz2), id(Wm))
    if wkey in _WCACHE:
        wd = _WCACHE[wkey]
    else:
        wd = _prep_weights(np.asarray(Wr), np.asarray(br), np.asarray(Ur),
                           np.asarray(bur), np.asarray(W), np.asarray(bw),
                           np.asarray(U), np.asarray(bu), np.asarray(Wz1),
                           np.asarray(bz1), np.asarray(Wz2), np.asarray(bz2),
                           np.asarray(Wm), np.asarray(bm))
        _WCACHE.clear()
        _WCACHE[wkey] = wd
    in_maps = []
    for k in range(NCORES):
        ft, qt, mt = _prep_core(facts, questions, prevM, k)
        in_maps.append(dict(facts_t=ft, q_t=qt, m_t=mt, **wd))
    nc = _get_nc()
    res = run_bass_kernel_spmd(nc, in_maps, core_ids=list(range(NCORES)))
    outs = [res.results[k]["out"] for k in range(NCORES)]
    full = np.concatenate(outs, axis=0)  # (128, 1024)
    return full[:, None, :].astype(np.float32)
